# revision 2
# baseline (speedup 1.0000x reference)
"""Block-sparse self-attention (block=20, heads=4) on 8 TRN2 NeuronCores, v3.

Data-parallel over batch B=32 -> 4 sequences/core; weights replicated.
Globally software-pipelined emission: per group-slot gi the engines see
  PE : [yproj(gi-3)] [scores+mask(gi)] [AV+den(gi-1)] [transpose(gi-2)]
  ACT: [exp(gi)] [qk copies of a future chunk's projection]
  DVE: [recip+normalize(gi-1)] [y copy(gi-3)]
  Pool:[v copies (future proj)] [o^T copy(gi-2)]
so every instruction's deps are ~1 slot old when it reaches its in-order
queue head. Projection units of chunk c+1 are spread across chunk c's
slots. Scores land in a packed 2-bank PSUM tile (bank = h%2 so the
row-packed concurrent score MMs never collide); a rank-7 additive mask
(2 PE matmuls, off-block -64) is accumulated pre-exp; ONE exp per group.
Softmax denominators are tiny N=1 matmuls into spare PSUM columns of the
scores tile. y is stored bf16 with one chunk-level DMA; host casts back.

All matmuls bf16 with fp32 PSUM accumulation. Wq is host-prescaled by
1/8 so q/k PSUM->SBUF moves are plain copies. Inputs consolidated into 3
DRAM tensors (xT, packed weights, packed consts).
"""

import numpy as np
import ml_dtypes

import concourse.bass as bass
import concourse.mybir as mybir
import concourse.tile as tile
from concourse import bacc
from concourse.bass_utils import run_bass_kernel_spmd

F32 = mybir.dt.float32
BF16 = mybir.dt.bfloat16

B, T, D = 32, 4000, 256
BS = 20            # attention block size
H = 4              # heads
HD = D // H        # 64
NCORES = 8
BPC = B // NCORES  # sequences per core

SUB = 120          # tokens per subtile (6 blocks)
GROUP = 1          # subtiles per packed-exp group (scps double-buffered)
DENC = 124         # den column base inside a 128-wide score slot

# const-pack column layout (bf16, [128, CPK])
_ID0 = 0           # identity [128, 128]
_ONEPP = 128       # ones col [128, 1]
_U7 = 129          # U7 [7, 120]
_V7 = 249          # V7 [7, 120]
_ONEROW = 369      # ones row [1, 120]
_BV = 489          # v bias row [1, 256]
_BY = 745          # y bias row [1, 256]
_MASK = 1001       # 0/1 mask [120, 120] (only for mask!=pe)
CPK = 1128

DEFAULT_CFG = dict(
    chunk=960,       # tokens per chunk
    mask="pe",       # pe | dve | pool
    qk_eng="act",    # engine for q/k PSUM->SBUF copies
    v_eng="dve",     # engine for v PSUM->SBUF copies
    y_eng="dve",     # engine for y PSUM->SBUF copies
    o_eng="dve",     # engine for o^T PSUM->SBUF copies (GPSIMD can't read PSUM)
    dA=1, dT=2, dY=4,  # pipeline depths: AV, transpose, yproj stage offsets
    scps="two",      # two | padded1 : score-PSUM layout
    scps_bufs=1,
)


def _chunks_for(t_total, chunk):
    out = []
    t0 = 0
    while t0 < t_total:
        ch = min(chunk, t_total - t0)
        subs = []
        off = 0
        while off < ch:
            subs.append(min(SUB, ch - off))
            off += subs[-1]
        out.append((t0, subs))
        t0 += ch
    return out


def build_program(bpc=BPC, t_total=T, vy_bias=False, qk_bias=False, cfg=None):
    cfg = dict(DEFAULT_CFG, **(cfg or {}))
    CH = cfg["chunk"]
    nc = bacc.Bacc("TRN2", target_bir_lowering=False, debug=False,
                   num_devices=NCORES)

    xT = nc.dram_tensor("xT", [bpc, D, t_total], BF16, kind="ExternalInput")
    wall = nc.dram_tensor("wall", [D, 1024], BF16, kind="ExternalInput")
    cpk = nc.dram_tensor("cpk", [128, CPK], BF16, kind="ExternalInput")
    if qk_bias:
        bqk = nc.dram_tensor("bqk", [4, 128], F32, kind="ExternalInput")
    y = nc.dram_tensor("y", [bpc, t_total, D], BF16, kind="ExternalOutput")

    xT_r = xT.rearrange("b (dk p) t -> b p dk t", p=128)
    wall_r = wall.rearrange("(dk p) e -> p dk e", p=128)

    def eng(name):
        return {"act": nc.scalar, "dve": nc.vector, "pool": nc.gpsimd}[name]

    _split_state = {}

    def copy_to(engine_name, dst, src):
        if engine_name == "split":
            n = _split_state.get("n", 0)
            _split_state["n"] = n + 1
            engine_name = "act" if n % 2 == 0 else "dve"
        if engine_name == "act":
            nc.scalar.activation(dst, src, mybir.ActivationFunctionType.Copy)
        else:
            eng(engine_name).tensor_copy(dst, src)

    with tile.TileContext(nc) as tc:
        with (
            tc.tile_pool(name="consts", bufs=1) as cpool,
            tc.tile_pool(name="xf", bufs=2) as xpool,
            tc.tile_pool(name="qk", bufs=2) as qkpool,
            tc.tile_pool(name="att", bufs=2) as apool,
            tc.tile_pool(name="out", bufs=2) as opool,
            tc.tile_pool(name="ps", bufs=1, space="PSUM") as pspool,
        ):
            cp = cpool.tile([128, CPK], BF16, tag="cpk")
            nc.sync.dma_start(out=cp, in_=cpk[:, :])
            wall_sb = cpool.tile([128, 2, 1024], BF16, tag="wall")
            nc.sync.dma_start(out=wall_sb, in_=wall_r)
            if qk_bias:
                bqk_sb = cpool.tile([128, 4], F32, tag="bqk")
                nc.sync.dma_start(out=bqk_sb, in_=bqk.rearrange("c p -> p c"))

            wqk = wall_sb[:, :, 0:512]
            wv = wall_sb[:, :, 512:768]
            wo = wall_sb[:, :, 768:1024]
            id_sb = cp[:, _ID0:_ID0 + 128]
            ones_pp = cp[:, _ONEPP:_ONEPP + 1]
            u7 = cp[0:7, _U7:_U7 + SUB]
            v7 = cp[0:7, _V7:_V7 + SUB]
            onesr = cp[0:1, _ONEROW:_ONEROW + SUB]
            bv_sb = cp[0:1, _BV:_BV + D]
            by_sb = cp[0:1, _BY:_BY + D]

            # ---------------- chunk / group descriptors ----------------
            chunks = []   # dicts: b, t0, subs, offs, state
            for b in range(bpc):
                for (t0, subs) in _chunks_for(t_total, CH):
                    offs = []
                    o = 0
                    for sz in subs:
                        offs.append(o)
                        o += sz
                    chunks.append(dict(b=b, t0=t0, subs=subs, offs=offs,
                                       ch=sum(subs)))
            groups = []   # (chunk_idx, si): one group per subtile
            for ci, c in enumerate(chunks):
                for si in range(len(c["subs"])):
                    groups.append((ci, si))
            # last group index per chunk (to emit the y DMA)
            last_group_of = {}
            for gi, (ci, g0) in enumerate(groups):
                last_group_of[ci] = gi

            # ---------------- projection units ----------------
            def proj_dma(ci):
                c = chunks[ci]
                xfm = xpool.tile([128, 2, CH], BF16, tag="xfm", bufs=3)
                c["xfm"] = xfm
                c["qk_sb"] = [None] * 4
                c["vt"] = None
                nc.sync.dma_start(
                    out=xfm[:, :, :c["ch"]],
                    in_=xT_r[c["b"], :, :, c["t0"]:c["t0"] + c["ch"]])

            def proj_pc(ci, pc):
                c = chunks[ci]
                ch = c["ch"]
                # PSUM split in 480-col bank-aligned segments (a single
                # matmul's output must stay within one 2KB PSUM bank)
                ps = pspool.tile([128, 2, 512], F32, tag="qkps", bufs=1)
                nseg = (ch + 479) // 480
                for g in range(nseg):
                    w = min(480, ch - g * 480)
                    for dk in range(2):
                        nc.tensor.matmul(
                            ps[:, g, :w],
                            wqk[:, dk, pc * 128:(pc + 1) * 128],
                            c["xfm"][:, dk, g * 480:g * 480 + w],
                            start=(dk == 0), stop=(dk == 1),
                        )
                sb = qkpool.tile([128, CH], BF16, tag=f"qk{pc}")
                if nseg == 1:
                    src = ps[:, 0, :ch]
                    dst = sb[:, :ch]
                else:
                    src = ps[:, :, :480]
                    dst = sb[:, :ch].rearrange("p (g w) -> p g w", g=nseg)
                if qk_bias:
                    nc.scalar.activation(
                        dst, src,
                        mybir.ActivationFunctionType.Identity,
                        bias=bqk_sb[:, pc:pc + 1], scale=1.0,
                    )
                else:
                    copy_to(cfg["qk_eng"], dst, src)
                c["qk_sb"][pc] = sb

            def proj_v(ci, si0):
                c = chunks[ci]
                if c["vt"] is None:
                    nsub = len(c["subs"])
                    vt = xpool.tile([SUB, nsub, H, HD + 1], BF16, tag="vt",
                                    bufs=3)
                    c["vt"] = vt
                    ones_b = bass.AP(
                        tensor=ones_pp.tensor, offset=ones_pp.offset,
                        ap=[ones_pp.ap[0][:], [0, nsub], [0, H], [0, 1]],
                    )[:SUB]
                    nc.vector.tensor_copy(vt[:, :, :, HD:HD + 1], ones_b)
                sis = [si0]
                if si0 + 1 < len(c["subs"]):
                    sis.append(si0 + 1)
                vps = pspool.tile([SUB, 2, D], F32, tag="vps", bufs=1)
                for j, si in enumerate(sis):
                    s = c["subs"][si]
                    off = c["offs"][si]
                    for dk in range(2):
                        nc.tensor.matmul(
                            vps[:s, j, :],
                            c["xfm"][:, dk, off:off + s],
                            wv[:, dk, :],
                            start=(dk == 0),
                            stop=(dk == 1 and not vy_bias),
                        )
                    if vy_bias:
                        nc.tensor.matmul(
                            vps[:s, j, :], onesr[:, :s], bv_sb[:, :],
                            start=False, stop=True,
                        )
                if len(sis) == 2 and c["subs"][sis[0]] == c["subs"][sis[1]]:
                    s = c["subs"][sis[0]]
                    copy_to(cfg["v_eng"],
                            c["vt"][:s, si0:si0 + 2, :, 0:HD],
                            vps[:s, :, :].rearrange(
                                "p j (h e) -> p j h e", h=H))
                else:
                    for j, si in enumerate(sis):
                        s = c["subs"][si]
                        copy_to(cfg["v_eng"],
                                c["vt"][:s, si, :, 0:HD],
                                vps[:s, j, :].rearrange(
                                    "p (h e) -> p h e", h=H))

            def proj_units(ci):
                units = [lambda ci=ci: proj_dma(ci)]
                for pc in range(4):
                    units.append(lambda ci=ci, pc=pc: proj_pc(ci, pc))
                for si in range(0, len(chunks[ci]["subs"]), 2):
                    units.append(lambda ci=ci, si=si: proj_v(ci, si))
                return units

            # ---------------- attention stages ----------------
            gstate = [dict() for _ in groups]

            def stage_scores(gi):
                ci, si = groups[gi]
                c = chunks[ci]
                s = c["subs"][si]
                st = gstate[gi]
                st["s"] = s
                qk_sb = c["qk_sb"]
                tw = slice(c["offs"][si], c["offs"][si] + s)
                if cfg["scps"] == "padded1":
                    # one tile, hp padded to a full bank stride: the
                    # row-packed concurrent MM pair lands in different banks
                    sc = pspool.tile([128, 2, 2, 128], F32, tag="scp",
                                     bufs=cfg["scps_bufs"],
                                     padded_shape=(None, None, 2, 256))
                    st["sc"] = (sc[:, 0], sc[:, 1])
                    st["sc1"] = sc
                else:
                    scA = pspool.tile([128, 2, 128], F32, tag="scA", bufs=1)
                    scB = pspool.tile([128, 2, 128], F32, tag="scB", bufs=1)
                    st["sc"] = (scA, scB)
                    st["sc1"] = None
                # start=True clears has_written for the WHOLE bank => only
                # the first MM per bank sets it; the other slot's bytes are
                # pending and get overwritten; mask MMs then accumulate.
                for hp in range(2):
                    sc = st["sc"][hp]
                    rp = hp * 64
                    for hc in range(2):
                        nc.tensor.matmul(
                            sc[:s, hc, :s],
                            qk_sb[2 + hc][rp:rp + 64, tw],
                            qk_sb[hc][rp:rp + 64, tw],
                            start=(hc == 0),
                            stop=(cfg["mask"] != "pe" and hc == 1),
                            tile_position=(rp, 0),
                            skip_group_check=True,
                        )
                if cfg["mask"] == "pe":
                    for hp in range(2):
                        sc = st["sc"][hp]
                        for hc in range(2):
                            nc.tensor.matmul(
                                sc[:s, hc, :s],
                                u7[:, :s], v7[:, :s],
                                start=False, stop=(hc == 1),
                                skip_group_check=True,
                            )

            def stage_exp(gi):
                st = gstate[gi]
                s = st["s"]
                ee = apool.tile([SUB, 2, 2, SUB], BF16, tag="ee", bufs=3)
                st["ee"] = ee
                if st["sc1"] is not None:
                    nc.scalar.activation(
                        ee[:s, :, :, :s], st["sc1"][:s, :, :, :s],
                        mybir.ActivationFunctionType.Exp)
                else:
                    for hp in range(2):
                        nc.scalar.activation(
                            ee[:s, hp, :, :s], st["sc"][hp][:s, :, :s],
                            mybir.ActivationFunctionType.Exp)
                if cfg["mask"] != "pe":
                    m = cp[0:s, _MASK:_MASK + s]
                    mb = bass.AP(
                        tensor=m.tensor, offset=m.offset,
                        ap=[m.ap[0][:], [0, 2], [0, 2], m.ap[1][:]],
                    )
                    e = nc.vector if cfg["mask"] == "dve" else nc.gpsimd
                    e.tensor_mul(ee[:s, :, :, :s], ee[:s, :, :, :s], mb)

            def stage_av(gi):
                ci, si = groups[gi]
                c = chunks[ci]
                st = gstate[gi]
                ee, s = st["ee"], st["s"]
                ops = pspool.tile([SUB, H, HD + 1], F32, tag="ops", bufs=1)
                for h in range(H):
                    nc.tensor.matmul(
                        ops[:s, h, :],
                        ee[:s, h % 2, h // 2, :s],
                        c["vt"][:s, si, h, :],
                        start=True, stop=True,
                    )
                rec = apool.tile([SUB, 4], F32, tag="rec")
                nc.vector.reciprocal(rec[:s, :].unsqueeze(2),
                                     ops[:s, :, HD:HD + 1])
                ov = opool.tile([SUB, H, HD], BF16, tag="ov", bufs=4)
                rec_b = bass.AP(
                    tensor=rec.tensor, offset=rec.offset,
                    ap=[rec.ap[0][:], [rec.ap[1][0], 4], [0, HD]],
                )[:s]
                nc.vector.tensor_mul(ov[:s, :, :], ops[:s, :, 0:HD], rec_b)
                st["ov"] = ov

            def stage_transp(gi):
                st = gstate[gi]
                s = st["s"]
                otps = pspool.tile([128, 2, SUB], BF16, tag="otps", bufs=1)
                ovf = st["ov"].rearrange("p a b -> p (a b)")
                for ec in range(2):
                    nc.tensor.transpose(
                        otps[:, ec, :s],
                        ovf[:s, ec * 128:(ec + 1) * 128],
                        id_sb[:s, :s])
                o_fm = opool.tile([128, 2, SUB], BF16, tag="osb", bufs=4)
                copy_to(cfg["o_eng"], o_fm[:, :, :s], otps[:, :, :s])
                st["ofm"] = o_fm

            def stage_yproj(gi):
                ci, si = groups[gi]
                c = chunks[ci]
                st = gstate[gi]
                s = st["s"]
                if c.get("ysb") is None:
                    c["ysb"] = opool.tile([SUB, len(c["subs"]), D], BF16,
                                          tag="ysb", name="ysb")
                ysb = c["ysb"]
                yps = pspool.tile([SUB, D], F32, tag="yps", bufs=1)
                for ec in range(2):
                    nc.tensor.matmul(
                        yps[:s, :],
                        st["ofm"][:, ec, :s],
                        wo[:, ec, :],
                        start=(ec == 0),
                        stop=(ec == 1 and not vy_bias),
                    )
                if vy_bias:
                    nc.tensor.matmul(
                        yps[:s, :], onesr[:, :s], by_sb[:, :],
                        start=False, stop=True,
                    )
                copy_to(cfg["y_eng"], ysb[:s, si, :], yps[:s, :])
                if last_group_of[ci] == gi:
                    subs, t0, b, ch = c["subs"], c["t0"], c["b"], c["ch"]
                    if all(sz == SUB for sz in subs):
                        ydst = y[b, t0:t0 + ch, :].rearrange(
                            "(si p) e -> p si e", p=SUB)
                        nc.sync.dma_start(out=ydst,
                                          in_=ysb[:, :len(subs), :])
                    else:
                        for sj, sz in enumerate(subs):
                            t0s = t0 + c["offs"][sj]
                            nc.sync.dma_start(out=y[b, t0s:t0s + sz, :],
                                              in_=ysb[:sz, sj, :])
                gstate[gi] = {}

            # ---------------- flattened pipelined emission ----------------
            # assign proj units of chunk ci (>=1) to group-slots of chunk ci-1
            slot_units = [[] for _ in groups]
            gslots_of_chunk = {}
            for gi, (ci, g0) in enumerate(groups):
                gslots_of_chunk.setdefault(ci, []).append(gi)
            for ci in range(1, len(chunks)):
                tslots = list(gslots_of_chunk.get(ci - 2, [])) + \
                    list(gslots_of_chunk[ci - 1])
                units = proj_units(ci)
                n = len(tslots)
                for k, u in enumerate(units):
                    # fill earliest slots first, round-robin
                    slot_units[tslots[k % n]].append(u)

            for u in proj_units(0):   # prologue
                u()
            NG = len(groups)
            dA, dT, dY = cfg["dA"], cfg["dT"], cfg["dY"]
            for gi in range(NG + dY):
                if gi < NG:
                    for u in slot_units[gi]:
                        u()
                if dY <= gi < NG + dY:
                    stage_yproj(gi - dY)
                if gi < NG:
                    stage_scores(gi)
                    stage_exp(gi)
                if dA <= gi < NG + dA:
                    stage_av(gi - dA)
                if dT <= gi < NG + dT:
                    stage_transp(gi - dT)

    nc.compile()
    return nc


_PROG = {}


def _get_program(bpc, t_total, vy_bias=False, qk_bias=False, cfg=None):
    key = (bpc, t_total, vy_bias, qk_bias, tuple(sorted((cfg or {}).items())))
    if key not in _PROG:
        _PROG[key] = build_program(bpc, t_total, vy_bias, qk_bias, cfg)
    return _PROG[key]


def _bf(a):
    return np.ascontiguousarray(a.astype(ml_dtypes.bfloat16))


def make_host_inputs(x, in_proj_w, in_proj_b, out_proj_w, out_proj_b,
                     cfg=None):
    cfg = dict(DEFAULT_CFG, **(cfg or {}))
    b_total, t_total, d = x.shape
    bpc = b_total // NCORES
    vy_bias = bool(np.any(in_proj_b[2 * D:]) or np.any(out_proj_b))
    qk_bias = bool(np.any(in_proj_b[:2 * D]))

    wq = in_proj_w[:D] * 0.125                  # host-prescaled q weights
    wk = in_proj_w[D:2 * D]
    wqkT = np.concatenate([wq, wk], axis=0).T   # [D, 512]
    wvT = in_proj_w[2 * D:].T                   # [D, 256]
    woT = out_proj_w.T                          # [D, 256]
    wall = _bf(np.concatenate([wqkT, wvT, woT], axis=1))  # [D, 1024]

    cpk = np.zeros((128, CPK), np.float32)
    cpk[:, _ID0:_ID0 + 128] = np.eye(128, dtype=np.float32)
    cpk[:, _ONEPP] = 1.0
    blk = np.arange(SUB) // BS
    onehot = (blk[:, None] == np.arange(6)[None, :]).astype(np.float32)
    cpk[0, _U7:_U7 + SUB] = 1.0
    cpk[1:7, _U7:_U7 + SUB] = onehot.T
    cpk[0, _V7:_V7 + SUB] = -64.0
    cpk[1:7, _V7:_V7 + SUB] = 64.0 * onehot.T
    cpk[0, _ONEROW:_ONEROW + SUB] = 1.0
    cpk[0, _BV:_BV + D] = in_proj_b[2 * D:]
    cpk[0, _BY:_BY + D] = out_proj_b
    _m = (blk[:, None] == blk[None, :]).astype(np.float32)
    cpk[:SUB, _MASK:_MASK + SUB] = _m
    cpk = _bf(cpk)

    in_maps = []
    for c in range(NCORES):
        xs = x[c * bpc:(c + 1) * bpc]
        xT = _bf(xs.transpose(0, 2, 1))
        m = {"xT": xT, "wall": wall, "cpk": cpk}
        if qk_bias:
            bqkh = np.ascontiguousarray(
                in_proj_b[:2 * D].reshape(4, 128).astype(np.float32))
            bqkh[:2] *= 0.125
            m["bqk"] = bqkh
        in_maps.append(m)
    return in_maps, bpc, t_total, vy_bias, qk_bias


def kernel(x, in_proj_w, in_proj_b, out_proj_w, out_proj_b, cfg=None):
    x = np.asarray(x, dtype=np.float32)
    in_proj_w = np.asarray(in_proj_w, dtype=np.float32)
    in_proj_b = np.asarray(in_proj_b, dtype=np.float32)
    out_proj_w = np.asarray(out_proj_w, dtype=np.float32)
    out_proj_b = np.asarray(out_proj_b, dtype=np.float32)

    in_maps, bpc, t_total, vy_bias, qk_bias = make_host_inputs(
        x, in_proj_w, in_proj_b, out_proj_w, out_proj_b, cfg)
    nc = _get_program(bpc, t_total, vy_bias, qk_bias, cfg)

    global _last_in_maps, _last_nc
    _last_in_maps = in_maps
    _last_nc = nc
    res = run_bass_kernel_spmd(nc, in_maps, core_ids=list(range(NCORES)))
    out = np.concatenate(
        [np.asarray(res.results[c]["y"]).astype(np.float32)
         for c in range(NCORES)], axis=0)
    return out


_last_in_maps = None
_last_nc = None
